# revision 2
# baseline (speedup 1.0000x reference)
"""Grouped GEMM (MoE routing) Trainium2 kernel.

Strategy: tensor-parallel shard of the output N dim across 8 NeuronCores.
Every core sees all T=8192 tokens and a 512-wide slice of every expert's
weights, so per-core work is identical regardless of segment sizes and a
single SPMD program (with the segment boundaries baked in as compile-time
constants) runs on all 8 cores.

Per core:  out_t[n, t] = sum_k w_t[e(t), k, n] * a_t[k, t]

Inputs and outputs are cast to bf16 on the host (rel err ~3e-3, far
under the 2e-2 gate).  Per-core HBM traffic ~105MB; two cores share one
716GB/s HBM stack, so the steady-state demand (2 x 230GB/s) fits but the
*startup transient* (first weight tile + first a blocks on all cores at
once) oversubscribes the stack and used to stall the PE stream for
~10us in the first 50us.  Mitigations:
  - apool bufs=2 (not 3): at most 2 a-blocks in flight, throttling the
    initial a burst from 11.7MB to 7.8MB.
  - all blocks after the first run nb-outer (ko inner): each 128-wide
    n-slab finishes its PSUM accumulation at 25/50/75/100% of the block,
    is cast to SBUF immediately and stored.  The nb0 store rides the
    sync (a) ring and the nb1-3 stores ride the scalar (w) ring, acting
    as in-order pacing gates: a(i+1) can only dispatch once block i is
    25% done, the next expert's weight chunks once 50-100% done.  This
    spreads DMA demand evenly instead of bursting at block boundaries.
  - nb-outer also shrinks the tail: after the last matmul only one
    128x119 cast + tiny store remain instead of the whole block's.
Matmul mapping: stationary lhsT = w tile [k=128, n=128], moving rhs =
a tile [k=128, tok<=512] in bf16, PSUM out [n=128, tok<=512] fp32.
Compute floor/core = T*K*NS/(128*128) cycles @2.4GHz = 437us.
A short burst of dummy matmuls on scratch SBUF warms the PE clock-gate
(HAM 1.2->2.4GHz) during the initial loads.
"""

import numpy as np
import ml_dtypes

import concourse.bacc as bacc
import concourse.bass as bass
import concourse.mybir as mybir
import concourse.tile as tile
from concourse.bass_utils import run_bass_kernel_spmd

NC = 8          # NeuronCores
P = 128         # partitions
TB = 512        # max token block (PSUM bank = 512 fp32)

BF16 = ml_dtypes.bfloat16

LAST_RESULT = {}


def _token_blocks(seg_starts, seg_ends):
    """Split each segment into balanced pieces of <=512 tokens, keeping all
    but at most one piece even-length (odd pieces misalign the bf16 k-chunk
    slices and cost ~0.15us/block)."""
    blocks = []  # (tstart, tlen, active_expert_idx)
    for widx, (s, t) in enumerate(zip(seg_starts, seg_ends)):
        ln = t - s
        npieces = max(1, -(-ln // TB))
        base2 = (ln // npieces) & ~1
        sizes = [base2] * npieces
        rem = ln - base2 * npieces
        for i in range(rem // 2):
            sizes[i] += 2
        if rem % 2:
            sizes[-1] += 1
        p = s
        for L in sizes:
            if L > 0:
                blocks.append((p, L, widx))
                p += L
    return blocks


WCH = 8         # ko per weight/a DMA chunk (4 chunks of 8KB+/partition)


def _build_program(T, K, NS, EA, blocks):
    f32 = mybir.dt.float32
    bf16 = mybir.dt.bfloat16
    KO = K // P
    NB = NS // P
    NCH = KO // WCH

    # per-ko stride padded to even so every k-chunk slice stays 4B-aligned
    CTA = sum(KO * (L + L % 2) for (_, L, _) in blocks)
    CTO = sum(NB * L for (_, L, _) in blocks)

    # group consecutive same-expert blocks into runs
    runs = []
    for blk in blocks:
        if runs and runs[-1][0] == blk[2]:
            runs[-1][1].append(blk)
        else:
            runs.append((blk[2], [blk]))

    nc = bacc.Bacc(None, target_bir_lowering=False)
    ab = nc.declare_dram_parameter("ab", [P, CTA], bf16, isOutput=False)
    wb = nc.declare_dram_parameter("wb", [EA, P, KO, NS], bf16, isOutput=False)
    ot = nc.declare_dram_parameter("ot", [P, CTO], bf16, isOutput=True)

    with tile.TileContext(nc) as tc:
        with (
            tc.tile_pool(name="wpool", bufs=3) as wpool,
            tc.tile_pool(name="apool", bufs=2) as apool,
            tc.tile_pool(name="opool", bufs=2) as opool,
            tc.tile_pool(name="psum", bufs=2, space=bass.MemorySpace.PSUM) as psum_pool,
        ):
            def load_w_range(wt, widx, s, e):
                nc.scalar.dma_start(
                    out=wt[:, s:e, :],
                    in_=wb[widx, :, s:e, :])

            # ko ranges: uniform 4-ko chunks for the first block/weights so
            # delivery stays ahead of the MM stream all the way through
            # block 0.  Steady-state blocks load in two 16-ko halves.
            FINE = [(4 * i, 4 * i + 4) for i in range(8)]
            COARSE = [(0, 16), (16, 32)]

            # PE pre-warm: dummy matmuls on scratch SBUF with no DMA deps
            # run during the initial load, so the HAM clock-gate opens
            # (1.2->2.4GHz takes ~3.4us of sustained PE work) before the
            # first real matmul issues.
            warm_w = wpool.tile([P, P], bf16, tag="warm", name="warm_w")
            warm_a = apool.tile([P, 192], bf16, tag="warm", name="warm_a")
            nc.vector.memset(warm_w[:, :], 0)
            nc.vector.memset(warm_a[:, :], 0)
            warm_ps = psum_pool.tile([P, 1, 192], f32, tag="ps", name="warm_ps",
                                     padded_shape=[P, NB, TB])
            for _ in range(24):
                nc.tensor.matmul(warm_ps[:, 0, :], warm_w[:, :], warm_a[:, :],
                                 start=True, stop=True)

            off_a = 0
            off_o = 0
            first_block = True
            w_next = wpool.tile([P, KO, NS], bf16, tag="w", name="w_tile")
            for (s, e) in FINE:
                load_w_range(w_next, runs[0][0], s, e)
            for ri, (widx, rblocks) in enumerate(runs):
                w_tile = w_next
                nbk = len(rblocks)
                if ri + 1 < len(runs):
                    w_next = wpool.tile([P, KO, NS], bf16, tag="w", name="w_tile")
                for bi, (ts, L, _) in enumerate(rblocks):
                    Lp = L + L % 2
                    a_tile = apool.tile([P, KO * Lp], bf16, tag="a", name="a_tile",
                                        padded_shape=[P, KO * TB])
                    for (s, e) in (FINE if first_block else COARSE):
                        nc.sync.dma_start(
                            out=a_tile[:, s * Lp:e * Lp],
                            in_=ab[:, off_a + s * Lp:off_a + e * Lp])
                    ptile = psum_pool.tile([P, NB, L], f32, tag="ps", name="ps",
                                           padded_shape=[P, NB, TB])
                    o_tile = opool.tile([P, NB * L], bf16, tag="o", name="o_tile",
                                        padded_shape=[P, NB * TB])
                    if first_block:
                        # ko-outer: consume each a/w chunk as it lands, so
                        # the startup transient never stalls mid-block.
                        for ko in range(KO):
                            for nb in range(NB):
                                nc.tensor.matmul(
                                    ptile[:, nb, :],
                                    w_tile[:, ko, nb * P:(nb + 1) * P],
                                    a_tile[:, ko * Lp:ko * Lp + L],
                                    start=(ko == 0),
                                    stop=(ko == KO - 1),
                                )
                        for nb in range(NB):
                            nc.vector.tensor_copy(o_tile[:, nb * L:(nb + 1) * L],
                                                  ptile[:, nb, :])
                        nc.scalar.dma_start(out=ot[:, off_o:off_o + NB * L],
                                            in_=o_tile[:, :])
                    else:
                        # nb-outer: each n-slab completes at 25/50/75/100%
                        # of the block; cast+store immediately.  Store nb0
                        # on the sync ring (gates the next a-block load at
                        # the 25% point), nb1-3 on the scalar ring (gates
                        # the paced next-expert weight chunks).
                        for nb in range(NB):
                            for ko in range(KO):
                                nc.tensor.matmul(
                                    ptile[:, nb, :],
                                    w_tile[:, ko, nb * P:(nb + 1) * P],
                                    a_tile[:, ko * Lp:ko * Lp + L],
                                    start=(ko == 0),
                                    stop=(ko == KO - 1),
                                )
                            nc.vector.tensor_copy(o_tile[:, nb * L:(nb + 1) * L],
                                                  ptile[:, nb, :])
                            eng = nc.sync if nb == 0 else nc.scalar
                            eng.dma_start(
                                out=ot[:, off_o + nb * L:off_o + (nb + 1) * L],
                                in_=o_tile[:, nb * L:(nb + 1) * L])
                    first_block = False
                    off_a += KO * Lp
                    off_o += NB * L
                    # pace the next expert's weight chunks across this run's
                    # blocks so the prefetch never bursts against the a-stream
                    if ri + 1 < len(runs):
                        c0 = bi * NCH // nbk
                        c1 = (bi + 1) * NCH // nbk
                        for c in range(c0, c1):
                            load_w_range(w_next, runs[ri + 1][0],
                                         c * WCH, (c + 1) * WCH)
    nc.compile()
    return nc


def kernel(a, b, c, seg_indptr, weight_indices, batch_size, **_):
    T, K = a.shape
    E, N, K2 = b.shape
    assert K == K2
    NS = N // NC
    KO = K // P
    NB = NS // P

    seg = np.asarray(seg_indptr).astype(np.int64)
    widx_arr = np.asarray(weight_indices).astype(np.int64)
    segs = [(int(seg[e]), int(seg[e + 1]), int(widx_arr[e]))
            for e in range(int(batch_size)) if seg[e + 1] > seg[e]]
    # process longest segments first: every expert switch is then covered by
    # a long compute run, hiding the next weight load entirely
    segs.sort(key=lambda s: s[0] - s[1])
    seg_starts = [s for s, _, _ in segs]
    seg_ends = [t for _, t, _ in segs]
    experts = [w for _, _, w in segs]
    EA = len(segs)
    blocks = _token_blocks(seg_starts, seg_ends)

    # a -> [P, KO, T] bf16 (partition-major k layout), then pack blocks so
    # each block is a [P, KO*L] slab with 32KB-contiguous partition lines.
    a = np.ascontiguousarray(a, dtype=np.float32)
    at_full = a.T.reshape(KO, P, T).transpose(1, 0, 2).astype(BF16)  # [P,KO,T]
    CTA = sum(KO * (L + L % 2) for (_, L, _) in blocks)
    ab_np = np.zeros((P, CTA), dtype=BF16)
    off = 0
    for (ts, L, _) in blocks:
        Lp = L + L % 2
        tmp = np.zeros((P, KO, Lp), dtype=BF16)
        tmp[:, :, :L] = at_full[:, :, ts:ts + L]
        ab_np[:, off:off + KO * Lp] = tmp.reshape(P, KO * Lp)
        off += KO * Lp

    # weights: full [E_active, P, KO, N] bf16 once, slice per core.
    wt_full = np.empty((EA, P, KO, N), dtype=BF16)
    for ei, e in enumerate(experts):
        wt_full[ei] = b[e].T.reshape(KO, P, N).transpose(1, 0, 2)

    in_maps = []
    for j in range(NC):
        w = np.ascontiguousarray(wt_full[:, :, :, j * NS:(j + 1) * NS])
        in_maps.append({"ab": ab_np, "wb": w})

    nc = _build_program(T, K, NS, EA, blocks)

    import os
    trace = bool(int(os.environ.get("BASS_KERNEL_TRACE", "0")))
    res = run_bass_kernel_spmd(nc, in_maps, list(range(NC)), trace=trace)
    LAST_RESULT["exec_time_ns"] = res.exec_time_ns
    LAST_RESULT["results"] = res

    out_t = np.empty((N, T), dtype=np.float32)
    for j in range(NC):
        otj = np.asarray(res.results[j]["ot"]).astype(np.float32)  # [P, CTO]
        off = 0
        for (ts, L, _) in blocks:
            blk = otj[:, off:off + NB * L].reshape(P, NB, L)
            out_t[j * NS:(j + 1) * NS, ts:ts + L] = (
                blk.transpose(1, 0, 2).reshape(NS, L))
            off += NB * L
    return np.ascontiguousarray(out_t.T)


# revision 4
# speedup vs baseline: 1.0220x; 1.0220x over previous
"""Grouped GEMM (MoE routing) Trainium2 kernel.

Strategy: tensor-parallel shard of the output N dim across 8 NeuronCores.
Every core sees all T=8192 tokens and a 512-wide slice of every expert's
weights, so per-core work is identical regardless of segment sizes and a
single SPMD program (with the segment boundaries baked in as compile-time
constants) runs on all 8 cores.

Per core:  out_t[n, t] = sum_k w_t[e(t), k, n] * a_t[k, t]

Inputs and outputs are cast to bf16 on the host (rel err ~3e-3, far
under the 2e-2 gate).  Per-core HBM traffic ~105MB; two cores share one
716GB/s HBM stack, so the steady-state demand (2 x 230GB/s) fits but the
*startup transient* (first weight tile + first a blocks on all cores at
once) oversubscribes the stack and used to stall the PE stream for
~10us in the first 50us.  Mitigations:
  - apool bufs=2 (not 3): at most 2 a-blocks in flight, throttling the
    initial a burst from 11.7MB to 7.8MB.
  - all blocks after the first run nb-outer (ko inner): each 128-wide
    n-slab finishes its PSUM accumulation at 25/50/75/100% of the block,
    is cast to SBUF immediately and stored.  The nb0 store rides the
    sync (a) ring and the nb1-3 stores ride the scalar (w) ring, acting
    as in-order pacing gates: a(i+1) can only dispatch once block i is
    25% done, the next expert's weight chunks once 50-100% done.  This
    spreads DMA demand evenly instead of bursting at block boundaries.
  - nb-outer also shrinks the tail: after the last matmul only one
    128x119 cast + tiny store remain instead of the whole block's.
Matmul mapping: stationary lhsT = w tile [k=128, n=128], moving rhs =
a tile [k=128, tok<=512] in bf16, PSUM out [n=128, tok<=512] fp32.
Compute floor/core = T*K*NS/(128*128) cycles @2.4GHz = 437us.
A short burst of dummy matmuls on scratch SBUF warms the PE clock-gate
(HAM 1.2->2.4GHz) during the initial loads.
"""

import numpy as np
import ml_dtypes

import concourse.bacc as bacc
import concourse.bass as bass
import concourse.mybir as mybir
import concourse.tile as tile
from concourse.bass_utils import run_bass_kernel_spmd

NC = 8          # NeuronCores
P = 128         # partitions
TB = 512        # max token block (PSUM bank = 512 fp32)

BF16 = ml_dtypes.bfloat16

LAST_RESULT = {}


def _token_blocks(seg_starts, seg_ends):
    """Split each segment into balanced pieces of <=512 tokens, keeping all
    but at most one piece even-length (odd pieces misalign the bf16 k-chunk
    slices and cost ~0.15us/block)."""
    blocks = []  # (tstart, tlen, active_expert_idx)
    for widx, (s, t) in enumerate(zip(seg_starts, seg_ends)):
        ln = t - s
        npieces = max(1, -(-ln // TB))
        base2 = (ln // npieces) & ~1
        sizes = [base2] * npieces
        rem = ln - base2 * npieces
        for i in range(rem // 2):
            sizes[i] += 2
        if rem % 2:
            sizes[-1] += 1
        p = s
        for L in sizes:
            if L > 0:
                blocks.append((p, L, widx))
                p += L
    return blocks


WCH = 8         # ko per weight/a DMA chunk (4 chunks of 8KB+/partition)


def _build_program(T, K, NS, EA, blocks):
    f32 = mybir.dt.float32
    bf16 = mybir.dt.bfloat16
    KO = K // P
    NB = NS // P
    NCH = KO // WCH

    # per-ko stride padded to even so every k-chunk slice stays 4B-aligned
    CTA = sum(KO * (L + L % 2) for (_, L, _) in blocks)
    CTO = sum(NB * L for (_, L, _) in blocks)

    # group consecutive same-expert blocks into runs
    runs = []
    for blk in blocks:
        if runs and runs[-1][0] == blk[2]:
            runs[-1][1].append(blk)
        else:
            runs.append((blk[2], [blk]))

    nc = bacc.Bacc(None, target_bir_lowering=False)
    ab = nc.declare_dram_parameter("ab", [P, CTA], bf16, isOutput=False)
    wb = nc.declare_dram_parameter("wb", [EA, P, KO, NS], bf16, isOutput=False)
    ot = nc.declare_dram_parameter("ot", [P, CTO], bf16, isOutput=True)

    with tile.TileContext(nc) as tc:
        with (
            tc.tile_pool(name="wpool", bufs=3) as wpool,
            tc.tile_pool(name="apool", bufs=3) as apool,
            tc.tile_pool(name="opool", bufs=2) as opool,
            tc.tile_pool(name="psum", bufs=2, space=bass.MemorySpace.PSUM) as psum_pool,
        ):
            def load_w_range(wt, widx, s, e):
                nc.scalar.dma_start(
                    out=wt[:, s:e, :],
                    in_=wb[widx, :, s:e, :])

            # ko ranges: uniform 4-ko chunks for the first block/weights so
            # delivery stays ahead of the MM stream all the way through
            # block 0.  Steady-state blocks load in two 16-ko halves.
            FINE = [(4 * i, 4 * i + 4) for i in range(8)]
            COARSE = [(0, 16), (16, 32)]

            # PE pre-warm: dummy matmuls on scratch SBUF with no DMA deps
            # run during the initial load, so the HAM clock-gate opens
            # (1.2->2.4GHz takes ~3.4us of sustained PE work) before the
            # first real matmul issues.
            warm_w = wpool.tile([P, P], bf16, tag="warm", name="warm_w")
            warm_a = apool.tile([P, 192], bf16, tag="warm", name="warm_a")
            nc.vector.memset(warm_w[:, :], 0)
            nc.vector.memset(warm_a[:, :], 0)
            warm_ps = psum_pool.tile([P, 1, 192], f32, tag="ps", name="warm_ps",
                                     padded_shape=[P, NB, TB])
            for _ in range(24):
                nc.tensor.matmul(warm_ps[:, 0, :], warm_w[:, :], warm_a[:, :],
                                 start=True, stop=True)

            off_a = 0
            off_o = 0
            first_block = True
            w_next = wpool.tile([P, KO, NS], bf16, tag="w", name="w_tile")
            for (s, e) in FINE:
                load_w_range(w_next, runs[0][0], s, e)
            for ri, (widx, rblocks) in enumerate(runs):
                w_tile = w_next
                nbk = len(rblocks)
                if ri + 1 < len(runs):
                    w_next = wpool.tile([P, KO, NS], bf16, tag="w", name="w_tile")
                for bi, (ts, L, _) in enumerate(rblocks):
                    Lp = L + L % 2
                    a_tile = apool.tile([P, KO * Lp], bf16, tag="a", name="a_tile",
                                        padded_shape=[P, KO * TB])
                    for (s, e) in (FINE if first_block else COARSE):
                        nc.sync.dma_start(
                            out=a_tile[:, s * Lp:e * Lp],
                            in_=ab[:, off_a + s * Lp:off_a + e * Lp])
                    ptile = psum_pool.tile([P, NB, L], f32, tag="ps", name="ps",
                                           padded_shape=[P, NB, TB])
                    o_tile = opool.tile([P, NB * L], bf16, tag="o", name="o_tile",
                                        padded_shape=[P, NB * TB])
                    if first_block:
                        # ko-outer: consume each a/w chunk as it lands, so
                        # the startup transient never stalls mid-block.
                        for ko in range(KO):
                            for nb in range(NB):
                                nc.tensor.matmul(
                                    ptile[:, nb, :],
                                    w_tile[:, ko, nb * P:(nb + 1) * P],
                                    a_tile[:, ko * Lp:ko * Lp + L],
                                    start=(ko == 0),
                                    stop=(ko == KO - 1),
                                )
                        for nb in range(NB):
                            nc.vector.tensor_copy(o_tile[:, nb * L:(nb + 1) * L],
                                                  ptile[:, nb, :])
                        nc.scalar.dma_start(out=ot[:, off_o:off_o + NB * L],
                                            in_=o_tile[:, :])
                    else:
                        # nb-outer: each n-slab completes at 25/50/75/100%
                        # of the block; cast+store immediately.  All stores
                        # ride the scalar ring, acting as in-order pacing
                        # gates for the next-expert weight chunks queued
                        # behind them; the sync ring stays a pure ungated
                        # a-stream so a(i+1) always lands a block early.
                        for nb in range(NB):
                            for ko in range(KO):
                                nc.tensor.matmul(
                                    ptile[:, nb, :],
                                    w_tile[:, ko, nb * P:(nb + 1) * P],
                                    a_tile[:, ko * Lp:ko * Lp + L],
                                    start=(ko == 0),
                                    stop=(ko == KO - 1),
                                )
                            nc.vector.tensor_copy(o_tile[:, nb * L:(nb + 1) * L],
                                                  ptile[:, nb, :])
                            nc.scalar.dma_start(
                                out=ot[:, off_o + nb * L:off_o + (nb + 1) * L],
                                in_=o_tile[:, nb * L:(nb + 1) * L])
                    first_block = False
                    off_a += KO * Lp
                    off_o += NB * L
                    # pace the next expert's weight chunks across this run's
                    # blocks so the prefetch never bursts against the a-stream
                    if ri + 1 < len(runs):
                        c0 = bi * NCH // nbk
                        c1 = (bi + 1) * NCH // nbk
                        for c in range(c0, c1):
                            load_w_range(w_next, runs[ri + 1][0],
                                         c * WCH, (c + 1) * WCH)
    nc.compile()
    return nc


def kernel(a, b, c, seg_indptr, weight_indices, batch_size, **_):
    T, K = a.shape
    E, N, K2 = b.shape
    assert K == K2
    NS = N // NC
    KO = K // P
    NB = NS // P

    seg = np.asarray(seg_indptr).astype(np.int64)
    widx_arr = np.asarray(weight_indices).astype(np.int64)
    segs = [(int(seg[e]), int(seg[e + 1]), int(widx_arr[e]))
            for e in range(int(batch_size)) if seg[e + 1] > seg[e]]
    # process longest segments first: every expert switch is then covered by
    # a long compute run, hiding the next weight load entirely
    segs.sort(key=lambda s: s[0] - s[1])
    seg_starts = [s for s, _, _ in segs]
    seg_ends = [t for _, t, _ in segs]
    experts = [w for _, _, w in segs]
    EA = len(segs)
    blocks = _token_blocks(seg_starts, seg_ends)

    # a -> [P, KO, T] bf16 (partition-major k layout), then pack blocks so
    # each block is a [P, KO*L] slab with 32KB-contiguous partition lines.
    a = np.ascontiguousarray(a, dtype=np.float32)
    at_full = a.T.reshape(KO, P, T).transpose(1, 0, 2).astype(BF16)  # [P,KO,T]
    CTA = sum(KO * (L + L % 2) for (_, L, _) in blocks)
    ab_np = np.zeros((P, CTA), dtype=BF16)
    off = 0
    for (ts, L, _) in blocks:
        Lp = L + L % 2
        tmp = np.zeros((P, KO, Lp), dtype=BF16)
        tmp[:, :, :L] = at_full[:, :, ts:ts + L]
        ab_np[:, off:off + KO * Lp] = tmp.reshape(P, KO * Lp)
        off += KO * Lp

    # weights: full [E_active, P, KO, N] bf16 once, slice per core.
    wt_full = np.empty((EA, P, KO, N), dtype=BF16)
    for ei, e in enumerate(experts):
        wt_full[ei] = b[e].T.reshape(KO, P, N).transpose(1, 0, 2)

    in_maps = []
    for j in range(NC):
        w = np.ascontiguousarray(wt_full[:, :, :, j * NS:(j + 1) * NS])
        in_maps.append({"ab": ab_np, "wb": w})

    nc = _build_program(T, K, NS, EA, blocks)

    import os
    trace = bool(int(os.environ.get("BASS_KERNEL_TRACE", "0")))
    res = run_bass_kernel_spmd(nc, in_maps, list(range(NC)), trace=trace)
    LAST_RESULT["exec_time_ns"] = res.exec_time_ns
    LAST_RESULT["results"] = res

    out_t = np.empty((N, T), dtype=np.float32)
    for j in range(NC):
        otj = np.asarray(res.results[j]["ot"]).astype(np.float32)  # [P, CTO]
        off = 0
        for (ts, L, _) in blocks:
            blk = otj[:, off:off + NB * L].reshape(P, NB, L)
            out_t[j * NS:(j + 1) * NS, ts:ts + L] = (
                blk.transpose(1, 0, 2).reshape(NS, L))
            off += NB * L
    return np.ascontiguousarray(out_t.T)


# revision 7
# speedup vs baseline: 1.0533x; 1.0307x over previous
"""Grouped GEMM (MoE routing) Trainium2 kernel.

Strategy: tensor-parallel shard of the output N dim across 8 NeuronCores.
Every core sees all T=8192 tokens and a 512-wide slice of every expert's
weights, so per-core work is identical regardless of segment sizes and a
single SPMD program (with the segment boundaries baked in as compile-time
constants) runs on all 8 cores.

Per core:  out_t[n, t] = sum_k w_t[e(t), k, n] * a_t[k, t]

Inputs and outputs are cast to bf16 on the host (rel err ~3e-3, far
under the 2e-2 gate), cutting HBM traffic to ~109MB/core, below the
~280GB/s effective per-core DMA roofline for the 437us compute span.
All DMAs are laid out so each SBUF partition line is one contiguous
HBM run.  a-block loads ride the sync HWDGE queue; weight loads +
output stores ride the scalar HWDGE queue.  Experts are processed in
descending segment-length order and the next expert's weight chunks
are paced across the current run's blocks, so prefetches never burst
against the a-stream at the shared HBM port.  A short burst of dummy
matmuls on scratch SBUF warms the PE clock-gate (HAM 1.2->2.4GHz)
during the initial loads.

Matmul mapping: stationary lhsT = w tile [k=128, n=128], moving rhs =
a tile [k=128, tok<=512] in bf16, PSUM out [n=128, tok<=512] fp32,
accumulated over the 32 k-chunks.  Compute floor/core = T*K*NS/(128*128)
cycles @2.4GHz = 437us; measured ~465us (framework preamble + teardown
account for most of the difference).
"""

import numpy as np
import ml_dtypes

import concourse.bacc as bacc
import concourse.bass as bass
import concourse.mybir as mybir
import concourse.tile as tile
from concourse.bass_utils import run_bass_kernel_spmd

NC = 8          # NeuronCores
P = 128         # partitions
TB = 512        # max token block (PSUM bank = 512 fp32)

BF16 = ml_dtypes.bfloat16

LAST_RESULT = {}


def _token_blocks(seg_starts, seg_ends):
    """Split each segment into balanced pieces of <=512 tokens, keeping all
    but at most one piece even-length (odd pieces misalign the bf16 k-chunk
    slices and cost ~0.15us/block)."""
    blocks = []  # (tstart, tlen, active_expert_idx)
    for widx, (s, t) in enumerate(zip(seg_starts, seg_ends)):
        ln = t - s
        npieces = max(1, -(-ln // TB))
        base2 = (ln // npieces) & ~1
        sizes = [base2] * npieces
        rem = ln - base2 * npieces
        for i in range(rem // 2):
            sizes[i] += 2
        if rem % 2:
            sizes[-1] += 1
        p = s
        for L in sizes:
            if L > 0:
                blocks.append((p, L, widx))
                p += L
    return blocks


WCH = 8         # ko per weight/a DMA chunk (4 chunks of 8KB+/partition)


def _build_program(T, K, NS, EA, blocks):
    f32 = mybir.dt.float32
    bf16 = mybir.dt.bfloat16
    KO = K // P
    NB = NS // P
    NCH = KO // WCH

    # per-ko stride padded to even so every k-chunk slice stays 4B-aligned
    CTA = sum(KO * (L + L % 2) for (_, L, _) in blocks)
    CTO = sum(NB * L for (_, L, _) in blocks)

    # group consecutive same-expert blocks into runs
    runs = []
    for blk in blocks:
        if runs and runs[-1][0] == blk[2]:
            runs[-1][1].append(blk)
        else:
            runs.append((blk[2], [blk]))

    nc = bacc.Bacc(None, target_bir_lowering=False)
    ab = nc.declare_dram_parameter("ab", [P, CTA], bf16, isOutput=False)
    wb = nc.declare_dram_parameter("wb", [EA, P, KO, NS], bf16, isOutput=False)
    ot = nc.declare_dram_parameter("ot", [P, CTO], bf16, isOutput=True)

    with tile.TileContext(nc) as tc:
        with (
            tc.tile_pool(name="wpool", bufs=3) as wpool,
            tc.tile_pool(name="apool", bufs=3) as apool,
            tc.tile_pool(name="opool", bufs=2) as opool,
            tc.tile_pool(name="psum", bufs=2, space=bass.MemorySpace.PSUM) as psum_pool,
        ):
            def load_w_range(wt, widx, s, e):
                nc.scalar.dma_start(
                    out=wt[:, s:e, :],
                    in_=wb[widx, :, s:e, :])

            # ko ranges: uniform 4-ko chunks for the first block/weights so
            # delivery stays ahead of the MM stream all the way through
            # block 0 (coarser mid-block chunks stall at their boundary;
            # finer leading chunks start MMs too early and stall the same
            # way - both measured +5-6us).  Steady-state blocks load in two
            # 16-ko halves (a single whole-block DMA measured +2us: its
            # delivery tail lands right when the block starts).
            FINE = [(4 * i, 4 * i + 4) for i in range(8)]
            COARSE = [(0, 16), (16, 32)]

            # PE pre-warm: dummy matmuls on scratch SBUF with no DMA deps.
            # The HAM clock-gate (1.2->2.4GHz) opens ~10.7us after the
            # first sustained PE activity, regardless of short gaps.  Size
            # the warmup to carry PE activity until the gate opens (~70
            # matmuls x 160ns at 1.2GHz), so block0's real matmuls run
            # entirely at 2.4GHz instead of spending their first half at
            # 1.2GHz.  Self-adaptive: on a warm clock the same warmup just
            # finishes in half the time and real matmuls start earlier.
            warm_w = wpool.tile([P, P], bf16, tag="warm", name="warm_w")
            warm_a = apool.tile([P, 192], bf16, tag="warm", name="warm_a")
            nc.vector.memset(warm_w[:, :], 0)
            nc.vector.memset(warm_a[:, :], 0)
            warm_ps = psum_pool.tile([P, 1, 192], f32, tag="ps", name="warm_ps",
                                     padded_shape=[P, NB, TB])
            for _ in range(70):
                nc.tensor.matmul(warm_ps[:, 0, :], warm_w[:, :], warm_a[:, :],
                                 start=True, stop=True)

            # flatten (run, block) structure; precompute per-block offsets
            flat = []  # (ri, bi, nbk, L)
            for ri, (widx, rblocks) in enumerate(runs):
                for bi, (ts, L, _) in enumerate(rblocks):
                    flat.append((ri, bi, len(rblocks), L))
            offs_a = []
            offs_o = []
            oa = oo = 0
            for (_, _, _, L) in flat:
                offs_a.append(oa)
                offs_o.append(oo)
                oa += KO * (L + L % 2)
                oo += NB * L

            a_tiles = {}

            def emit_a_load(idx, eng, ranges):
                L = flat[idx][3]
                Lp = L + L % 2
                at = apool.tile([P, KO * Lp], bf16, tag="a", name="a_tile",
                                padded_shape=[P, KO * TB])
                for (s, e) in ranges:
                    eng.dma_start(
                        out=at[:, s * Lp:e * Lp],
                        in_=ab[:, offs_a[idx] + s * Lp:offs_a[idx] + e * Lp])
                a_tiles[idx] = at

            nblk = len(flat)
            w_next = wpool.tile([P, KO, NS], bf16, tag="w", name="w_tile")
            for (s, e) in FINE:
                load_w_range(w_next, runs[0][0], s, e)
            w_tile = None
            for i, (ri, bi, nbk, L) in enumerate(flat):
                if bi == 0:
                    w_tile = w_next
                    if ri + 1 < len(runs):
                        w_next = wpool.tile([P, KO, NS], bf16, tag="w",
                                            name="w_tile")
                Lp = L + L % 2
                if i not in a_tiles:
                    emit_a_load(i, nc.sync, FINE if i == 0 else COARSE)
                a_tile = a_tiles.pop(i)
                off_o = offs_o[i]
                ptile = psum_pool.tile([P, NB, L], f32, tag="ps", name="ps",
                                       padded_shape=[P, NB, TB])
                for ko in range(KO):
                    for nb in range(NB):
                        nc.tensor.matmul(
                            ptile[:, nb, :],
                            w_tile[:, ko, nb * P:(nb + 1) * P],
                            a_tile[:, ko * Lp:ko * Lp + L],
                            start=(ko == 0),
                            stop=(ko == KO - 1),
                        )
                o_tile = opool.tile([P, NB * L], bf16, tag="o", name="o_tile",
                                    padded_shape=[P, NB * TB])
                for nb in range(NB):
                    nc.vector.tensor_copy(o_tile[:, nb * L:(nb + 1) * L],
                                          ptile[:, nb, :])
                if i == nblk - 1:
                    # split the final store across both rings: its ~128
                    # small descriptors per half drain in parallel, halving
                    # the post-last-matmul tail
                    half = (NB // 2) * L
                    nc.scalar.dma_start(out=ot[:, off_o:off_o + half],
                                        in_=o_tile[:, :half])
                    nc.sync.dma_start(out=ot[:, off_o + half:off_o + NB * L],
                                      in_=o_tile[:, half:NB * L])
                else:
                    nc.scalar.dma_start(out=ot[:, off_o:off_o + NB * L],
                                        in_=o_tile[:, :])
                if i == 0 and nblk > 3:
                    # block2's a-load rides the scalar ring *behind* block0's
                    # store: it can only dispatch once block0 is done, which
                    # keeps the startup HBM burst down to w0+a0+a1 (11.8MB)
                    # -- what the shared stack can actually deliver while
                    # block0 computes.  Still lands a full block before
                    # block2 needs it.
                    emit_a_load(2, nc.scalar, COARSE)
                # pace the next expert's weight chunks across this run's
                # blocks so the prefetch never bursts against the a-stream
                if ri + 1 < len(runs):
                    c0 = bi * NCH // nbk
                    c1 = (bi + 1) * NCH // nbk
                    for c in range(c0, c1):
                        load_w_range(w_next, runs[ri + 1][0],
                                     c * WCH, (c + 1) * WCH)
    nc.compile()
    return nc


def kernel(a, b, c, seg_indptr, weight_indices, batch_size, **_):
    T, K = a.shape
    E, N, K2 = b.shape
    assert K == K2
    NS = N // NC
    KO = K // P
    NB = NS // P

    seg = np.asarray(seg_indptr).astype(np.int64)
    widx_arr = np.asarray(weight_indices).astype(np.int64)
    segs = [(int(seg[e]), int(seg[e + 1]), int(widx_arr[e]))
            for e in range(int(batch_size)) if seg[e + 1] > seg[e]]
    # process longest segments first: every expert switch is then covered by
    # a long compute run, hiding the next weight load entirely
    segs.sort(key=lambda s: s[0] - s[1])
    seg_starts = [s for s, _, _ in segs]
    seg_ends = [t for _, t, _ in segs]
    experts = [w for _, _, w in segs]
    EA = len(segs)
    blocks = _token_blocks(seg_starts, seg_ends)

    # a -> [P, KO, T] bf16 (partition-major k layout), then pack blocks so
    # each block is a [P, KO*L] slab with 32KB-contiguous partition lines.
    a = np.ascontiguousarray(a, dtype=np.float32)
    at_full = a.T.reshape(KO, P, T).transpose(1, 0, 2).astype(BF16)  # [P,KO,T]
    CTA = sum(KO * (L + L % 2) for (_, L, _) in blocks)
    ab_np = np.zeros((P, CTA), dtype=BF16)
    off = 0
    for (ts, L, _) in blocks:
        Lp = L + L % 2
        tmp = np.zeros((P, KO, Lp), dtype=BF16)
        tmp[:, :, :L] = at_full[:, :, ts:ts + L]
        ab_np[:, off:off + KO * Lp] = tmp.reshape(P, KO * Lp)
        off += KO * Lp

    # weights: full [E_active, P, KO, N] bf16 once, slice per core.
    wt_full = np.empty((EA, P, KO, N), dtype=BF16)
    for ei, e in enumerate(experts):
        wt_full[ei] = b[e].T.reshape(KO, P, N).transpose(1, 0, 2)

    in_maps = []
    for j in range(NC):
        w = np.ascontiguousarray(wt_full[:, :, :, j * NS:(j + 1) * NS])
        in_maps.append({"ab": ab_np, "wb": w})

    nc = _build_program(T, K, NS, EA, blocks)

    import os
    trace = bool(int(os.environ.get("BASS_KERNEL_TRACE", "0")))
    res = run_bass_kernel_spmd(nc, in_maps, list(range(NC)), trace=trace)
    LAST_RESULT["exec_time_ns"] = res.exec_time_ns
    LAST_RESULT["results"] = res

    out_t = np.empty((N, T), dtype=np.float32)
    for j in range(NC):
        otj = np.asarray(res.results[j]["ot"]).astype(np.float32)  # [P, CTO]
        off = 0
        for (ts, L, _) in blocks:
            blk = otj[:, off:off + NB * L].reshape(P, NB, L)
            out_t[j * NS:(j + 1) * NS, ts:ts + L] = (
                blk.transpose(1, 0, 2).reshape(NS, L))
            off += NB * L
    return np.ascontiguousarray(out_t.T)

